# revision 3
# baseline (speedup 1.0000x reference)
import sys

if "/opt/trn_rl_repo" not in sys.path:
    sys.path.insert(0, "/opt/trn_rl_repo")
import numpy as np

# PolyVGG16: 13 kerv2d layers (y=conv3x3(x,w,pad=1); out=(y+1)^2 + b), avgpool2x2
# after layers {1,3,6,9,12}, flatten -> FC [512->100].
# Data parallel: 256 imgs -> 8 cores x 32 imgs. Weights replicated.

LAYERS = [
    (3, 64, 32, 32, False),
    (64, 64, 32, 32, True),
    (64, 128, 16, 16, False),
    (128, 128, 16, 16, True),
    (128, 256, 8, 8, False),
    (256, 256, 8, 8, False),
    (256, 256, 8, 8, True),
    (256, 512, 4, 4, False),
    (512, 512, 4, 4, False),
    (512, 512, 4, 4, True),
    (512, 512, 2, 2, False),
    (512, 512, 2, 2, False),
    (512, 512, 2, 2, True),
]
B_CORE = 32
N_CORES = 8
GROUP_IMGS = {16: 2, 8: 8, 4: 32, 2: 32}

_CACHE = {}


def _chunks(c):
    n = max(1, c // 128)
    return n, c // n


def _build():
    from concourse import bacc, tile
    from concourse.mybir import dt, ActivationFunctionType as AFT, AluOpType as ALU
    from contextlib import ExitStack

    nc = bacc.Bacc("TRN2", target_bir_lowering=False, debug=False)
    fp16 = dt.float16
    f32 = dt.float32

    x0_d = nc.dram_tensor("x0", [27, B_CORE, 1024], fp16, kind="ExternalInput").ap()
    w_d = [nc.dram_tensor("w0", [27, 64], fp16, kind="ExternalInput").ap()]
    b_d = [nc.dram_tensor("b0", [64, 1], f32, kind="ExternalInput").ap()]
    for i in range(1, 13):
        cin, cout, H, W, pool = LAYERS[i]
        kcn, Kc = _chunks(cin)
        mcn, Mc = _chunks(cout)
        w_d.append(nc.dram_tensor(f"w{i}", [kcn, Kc, 9 * mcn * Mc], fp16,
                                  kind="ExternalInput").ap())
        b_d.append(nc.dram_tensor(f"b{i}", [cout, 1], f32, kind="ExternalInput").ap())
    wfc_d = nc.dram_tensor("wfc", [4, 128, 100], fp16, kind="ExternalInput").ap()
    bfc_d = nc.dram_tensor("bfc", [100, 1], f32, kind="ExternalInput").ap()
    out_d = nc.dram_tensor("out", [100, B_CORE], f32, kind="ExternalOutput").ap()

    with tile.TileContext(nc) as tc:
        with ExitStack() as ctx:
            sb = ctx.enter_context(tc.tile_pool(name="sb", bufs=1))
            pp = ctx.enter_context(tc.tile_pool(name="pp", bufs=1, space="PSUM"))

            def alloc_act(i, pfx):
                cin = LAYERS[i][0]
                H, W = LAYERS[i][2], LAYERS[i][3]
                kcn, Kc = _chunks(cin)
                fam = "e" if i % 2 == 0 else "o"
                tiles = []
                for k in range(kcn):
                    t = sb.tile([Kc, B_CORE, H + 2, W + 2], fp16,
                                tag=f"{fam}_k{k}", name=f"{pfx}_k{k}")
                    nc.vector.memset(t[:, :, 0, :], 0.0)
                    nc.vector.memset(t[:, :, H + 1, :], 0.0)
                    nc.vector.memset(t[:, :, 1:H + 1, 0], 0.0)
                    nc.vector.memset(t[:, :, 1:H + 1, W + 1], 0.0)
                    tiles.append(t)
                return tiles

            # ---- L0 + L1 weights / biases ----
            w0_sb = sb.tile([27, 64], fp16, tag="w0")
            nc.sync.dma_start(w0_sb[:], w_d[0][:, :])
            b0_sb = sb.tile([64, 1], f32, tag="b_l0")
            nc.sync.dma_start(b0_sb[:], b_d[0][:, :])
            w1_sb = sb.tile([64, 9 * 64], fp16, tag="w_k0", bufs=2, name="w1")
            nc.sync.dma_start(w1_sb[:], w_d[1][0, :, :])
            b1_sb = sb.tile([64, 1], f32, tag="b_m0", bufs=2, name="b1")
            nc.sync.dma_start(b1_sb[:], b_d[1][:, :])

            l2in = alloc_act(2, "L2in")

            # ---- L0 + L1, quarter-batch interleaved (8 imgs per quarter) ----
            for q in range(4):
                l1t = sb.tile([64, 8, 34, 34], fp16, tag="l1in", name=f"l1t_q{q}")
                nc.vector.memset(l1t[:, :, 0, :], 0.0)
                nc.vector.memset(l1t[:, :, 33, :], 0.0)
                nc.vector.memset(l1t[:, :, 1:33, 0], 0.0)
                nc.vector.memset(l1t[:, :, 1:33, 33], 0.0)
                for p in range(4):
                    xr = sb.tile([27, 2, 1024], fp16, tag="x0", bufs=2,
                                 name=f"x0_q{q}p{p}")
                    i0 = q * 8 + 2 * p
                    nc.sync.dma_start(xr[:], x0_d[:, i0:i0 + 2, :])
                    for j in range(2):
                        il = 2 * p + j
                        for h in range(2):
                            ps = pp.tile([64, 512], f32, tag="ps", bufs=6,
                                         name=f"ps_l0_q{q}i{il}h{h}")
                            nc.tensor.matmul(ps[:], w0_sb[:, 0:64],
                                             xr[:, j, h * 512:(h + 1) * 512],
                                             start=True, stop=True)
                            scr = sb.tile([64, 512], f32, tag="scr", bufs=2,
                                          name=f"scr_l0_q{q}i{il}h{h}")
                            nc.scalar.activation(scr[:], ps[:], AFT.Square, bias=1.0)
                            nc.vector.tensor_scalar(
                                l1t[:, il, 1 + 16 * h:17 + 16 * h, 1:33],
                                scr[:], b0_sb[:, 0:1], None, ALU.add)
                # L1 for this quarter's 8 images
                for il in range(8):
                    for h in range(2):
                        ps = pp.tile([64, 512], f32, tag="ps", bufs=6,
                                     name=f"ps_l1_q{q}i{il}h{h}")
                        for t in range(9):
                            a, bb = t // 3, t % 3
                            nc.tensor.matmul(
                                ps[:], w1_sb[:, t * 64:(t + 1) * 64],
                                l1t[:, il, a + 16 * h:a + 16 * h + 16, bb:bb + 32],
                                start=(t == 0), stop=(t == 8))
                        scr4 = sb.tile([64, 16, 16, 2], f32, tag="scr", bufs=2,
                                       name=f"s4_l1_q{q}i{il}h{h}")
                        nc.scalar.activation(scr4[:], ps[:], AFT.Square, bias=1.0)
                        t1 = sb.tile([64, 8, 2, 16], f32, tag="t1", bufs=2,
                                     name=f"t1_l1_q{q}i{il}h{h}")
                        nc.vector.tensor_tensor(t1[:], scr4[:, :, :, 0],
                                                scr4[:, :, :, 1], ALU.add)
                        t2 = sb.tile([64, 8, 16], f32, tag="t2", bufs=2,
                                     name=f"t2_l1_q{q}i{il}h{h}")
                        nc.vector.tensor_tensor(t2[:], t1[:, :, 0, :],
                                                t1[:, :, 1, :], ALU.add)
                        nc.vector.tensor_scalar(
                            l2in[0][:, q * 8 + il, 1 + 8 * h:9 + 8 * h, 1:17],
                            t2[:], 0.25, b1_sb[:, 0:1], ALU.mult, ALU.add)

            # ---- generic layers 2..12 ----
            cur = l2in
            fcin = None
            for i in range(2, 13):
                cin, cout, H, W, pool = LAYERS[i]
                kcn, Kc = _chunks(cin)
                mcn, Mc = _chunks(cout)
                n = GROUP_IMGS[H]
                ngroups = B_CORE // n
                N = n * H * W
                wts = []
                for k in range(kcn):
                    wt = sb.tile([Kc, 9 * mcn * Mc], fp16, tag=f"w_k{k}", bufs=2,
                                 name=f"w{i}_k{k}")
                    nc.sync.dma_start(wt[:], w_d[i][k, :, :])
                    wts.append(wt)
                bs = []
                for m in range(mcn):
                    bt = sb.tile([Mc, 1], f32, tag=f"b_m{m}", bufs=2,
                                 name=f"b{i}_m{m}")
                    nc.sync.dma_start(bt[:], b_d[i][m * Mc:(m + 1) * Mc, :])
                    bs.append(bt)
                if i == 12:
                    fcin = [sb.tile([128, B_CORE], fp16, tag=f"fc_k{m}",
                                    name=f"fcin_k{m}") for m in range(4)]
                    dest = fcin
                else:
                    dest = alloc_act(i + 1, f"L{i + 1}in")
                last = 9 * kcn - 1
                for g in range(ngroups):
                    i0 = g * n
                    for m in range(mcn):
                        ps = pp.tile([Mc, N], f32, tag="ps", bufs=6,
                                     name=f"ps_l{i}_g{g}m{m}")
                        cnt = 0
                        for t in range(9):
                            a, bb = t // 3, t % 3
                            for k in range(kcn):
                                nc.tensor.matmul(
                                    ps[:],
                                    wts[k][:, (t * mcn + m) * Mc:(t * mcn + m + 1) * Mc],
                                    cur[k][:, i0:i0 + n, a:a + H, bb:bb + W],
                                    start=(cnt == 0), stop=(cnt == last))
                                cnt += 1
                        if not pool:
                            scr = sb.tile([Mc, N], f32, tag="scr", bufs=2,
                                          name=f"scr_l{i}_g{g}m{m}")
                            nc.scalar.activation(scr[:], ps[:], AFT.Square, bias=1.0)
                            nc.vector.tensor_scalar(
                                dest[m][:, i0:i0 + n, 1:1 + H, 1:1 + W],
                                scr[:], bs[m][:, 0:1], None, ALU.add)
                        else:
                            H2, W2 = H // 2, W // 2
                            scr4 = sb.tile([Mc, n * H, W2, 2], f32, tag="scr",
                                           bufs=2, name=f"s4_l{i}_g{g}m{m}")
                            nc.scalar.activation(scr4[:], ps[:], AFT.Square, bias=1.0)
                            t1 = sb.tile([Mc, n * H // 2, 2, W2], f32, tag="t1",
                                         bufs=2, name=f"t1_l{i}_g{g}m{m}")
                            nc.vector.tensor_tensor(t1[:], scr4[:, :, :, 0],
                                                    scr4[:, :, :, 1], ALU.add)
                            t2 = sb.tile([Mc, n * H2 * W2], f32, tag="t2", bufs=2,
                                         name=f"t2_l{i}_g{g}m{m}")
                            nc.vector.tensor_tensor(t2[:], t1[:, :, 0, :],
                                                    t1[:, :, 1, :], ALU.add)
                            if i == 12:
                                dslice = dest[m][:, i0:i0 + n]
                            else:
                                dslice = dest[m][:, i0:i0 + n, 1:1 + H2, 1:1 + W2]
                            nc.vector.tensor_scalar(dslice, t2[:], 0.25,
                                                    bs[m][:, 0:1], ALU.mult, ALU.add)
                cur = dest

            # ---- FC ----
            wfc_sb = [sb.tile([128, 100], fp16, tag=f"wfc_k{k}", name=f"wfc{k}")
                      for k in range(4)]
            for k in range(4):
                nc.sync.dma_start(wfc_sb[k][:], wfc_d[k, :, :])
            bfc_sb = sb.tile([100, 1], f32, tag="bfc")
            nc.sync.dma_start(bfc_sb[:], bfc_d[:, :])
            psf = pp.tile([100, B_CORE], f32, tag="psfc")
            for k in range(4):
                nc.tensor.matmul(psf[:], wfc_sb[k][:], fcin[k][:],
                                 start=(k == 0), stop=(k == 3))
            out_sb = sb.tile([100, B_CORE], f32, tag="outsb")
            nc.scalar.activation(out_sb[:], psf[:], AFT.Identity,
                                 bias=bfc_sb[:, 0:1])
            nc.sync.dma_start(out_d[:], out_sb[:])

    nc.compile()
    return nc


def _prep_inputs(x, params):
    xp = np.zeros((256, 3, 34, 34), np.float16)
    xp[:, :, 1:33, 1:33] = np.asarray(x, np.float32).astype(np.float16)
    S = np.empty((27, 256, 1024), np.float16)
    for t in range(9):
        a, bb = t // 3, t % 3
        blk = xp[:, :, a:a + 32, bb:bb + 32]
        S[t * 3:(t + 1) * 3] = blk.transpose(1, 0, 2, 3).reshape(3, 256, 1024)

    base = {}
    w0 = np.asarray(params["w0"], np.float32)
    base["w0"] = np.ascontiguousarray(w0.transpose(2, 3, 1, 0)).reshape(27, 64).astype(np.float16)
    base["b0"] = np.asarray(params["b0"], np.float32).reshape(64, 1)
    for i in range(1, 13):
        cin, cout = LAYERS[i][0], LAYERS[i][1]
        kcn, Kc = _chunks(cin)
        mcn, Mc = _chunks(cout)
        w = np.asarray(params[f"w{i}"], np.float32)
        wt = np.ascontiguousarray(w.transpose(1, 2, 3, 0))  # [cin,3,3,cout]
        wt = wt.reshape(kcn, Kc, 9, mcn, Mc).reshape(kcn, Kc, 9 * mcn * Mc)
        base[f"w{i}"] = wt.astype(np.float16)
        base[f"b{i}"] = np.asarray(params[f"b{i}"], np.float32).reshape(cout, 1)
    base["wfc"] = np.ascontiguousarray(np.asarray(params["wfc"], np.float32).T) \
        .reshape(4, 128, 100).astype(np.float16)
    base["bfc"] = np.asarray(params["bfc"], np.float32).reshape(100, 1)

    in_maps = []
    for c in range(N_CORES):
        m = dict(base)
        m["x0"] = np.ascontiguousarray(S[:, c * B_CORE:(c + 1) * B_CORE, :])
        in_maps.append(m)
    return in_maps


def _get_runner():
    if "run" in _CACHE:
        return _CACHE["run"]
    import jax
    from jax.sharding import Mesh, PartitionSpec
    from jax.experimental.shard_map import shard_map
    from concourse import bass2jax, mybir

    nc = _build()
    _CACHE["nc"] = nc
    bass2jax.install_neuronx_cc_hook()
    partition_name = (
        nc.partition_id_tensor.name if nc.partition_id_tensor else None
    )
    in_names, out_names, out_avals, zero_outs = [], [], [], []
    for alloc in nc.m.functions[0].allocations:
        if not isinstance(alloc, mybir.MemoryLocationSet):
            continue
        name = alloc.memorylocations[0].name
        if alloc.kind == "ExternalInput":
            if name != partition_name:
                in_names.append(name)
        elif alloc.kind == "ExternalOutput":
            out_names.append(name)
            shape = tuple(alloc.tensor_shape)
            dtype = mybir.dt.np(alloc.dtype)
            out_avals.append(jax.core.ShapedArray(shape, dtype))
            zero_outs.append(np.zeros(shape, dtype))
    n_params = len(in_names)
    all_in_names = list(in_names) + list(out_names)
    if partition_name is not None:
        all_in_names.append(partition_name)

    def _body(*args):
        operands = list(args)
        if partition_name is not None:
            operands.append(bass2jax.partition_id_tensor())
        outs = bass2jax._bass_exec_p.bind(
            *operands,
            out_avals=tuple(out_avals),
            in_names=tuple(all_in_names),
            out_names=tuple(out_names),
            lowering_input_output_aliases=(),
            sim_require_finite=True,
            sim_require_nnan=True,
            nc=nc,
        )
        return tuple(outs)

    devices = jax.devices()[:N_CORES]
    mesh = Mesh(np.asarray(devices), ("core",))
    in_specs = (PartitionSpec("core"),) * (n_params + len(out_names))
    out_specs = (PartitionSpec("core"),) * len(out_names)
    sharded = jax.jit(
        shard_map(_body, mesh=mesh, in_specs=in_specs, out_specs=out_specs,
                  check_rep=False),
        keep_unused=True,
    )

    def run(in_maps):
        per_core = [[np.asarray(m[name]) for name in in_names] for m in in_maps]
        args = [
            np.concatenate([per_core[c][i] for c in range(N_CORES)], axis=0)
            for i in range(n_params)
        ]
        args += [np.concatenate([z] * N_CORES, axis=0) for z in zero_outs]
        out = jax.block_until_ready(sharded(*args))
        full = np.asarray(out[0])  # "out": [8*100, 32]
        per = np.split(full, N_CORES, axis=0)
        return per

    _CACHE["run"] = run
    return run


def kernel(x, params):
    run = _get_runner()
    in_maps = _prep_inputs(x, params)
    per = run(in_maps)
    return np.concatenate([p.T for p in per], axis=0).astype(np.float32)
